# revision 1
# baseline (speedup 1.0000x reference)
"""v4: monolithic bf16 attention on 8 trn2 NeuronCores (no collectives).

Core c handles batch c//2, query half c%2, loading the batch's full K and V.
All inputs are cast fp32->bf16 during the SWDGE DMA load using contiguous
per-partition access (each partition holds consecutive sequence rows; the
resulting sequence permutation cancels in softmax/attnV for K/V and is
undone in the output DMA for Q). PE transposes and matmuls run bf16 at
1 cycle/row with fp32 PSUM accumulation. Scores are computed in the [k, q]
layout (softmax free-dim never reduced: ones-column in the attnV stationary
yields row-sums; normalization deferred to the output epilogue).
"""

import sys

if "/opt/trn_rl_repo" not in sys.path:
    sys.path.insert(0, "/opt/trn_rl_repo")

import numpy as np

N, L, H, D = 4, 2048, 1024, 64
QSH = L // 2
NCORES = 8
HC = H // 128
NRNG = L // 512


def build_bass():
    import concourse.bass as bass
    import concourse.mybir as mybir
    from concourse import bacc
    from concourse.masks import make_identity
    from concourse.tile import TileContext

    f32 = mybir.dt.float32
    bf16 = mybir.dt.bfloat16
    AF = mybir.ActivationFunctionType

    nc = bacc.Bacc("TRN2", target_bir_lowering=False, debug=False)
    q_d = nc.dram_tensor("q", [QSH, H], f32, kind="ExternalInput").ap()
    k_d = nc.dram_tensor("k", [L, H], f32, kind="ExternalInput").ap()
    v_d = nc.dram_tensor("v", [L, H], f32, kind="ExternalInput").ap()
    wq_d = nc.dram_tensor("wq", [H, D], f32, kind="ExternalInput").ap()
    wk_d = nc.dram_tensor("wk", [H, D], f32, kind="ExternalInput").ap()
    wv_d = nc.dram_tensor("wv", [H, D], f32, kind="ExternalInput").ap()
    bq_d = nc.dram_tensor("bq8", [D, 1], f32, kind="ExternalInput").ap()
    bk_d = nc.dram_tensor("bk", [D, 1], f32, kind="ExternalInput").ap()
    bv_d = nc.dram_tensor("bv", [D, 1], f32, kind="ExternalInput").ap()
    out_d = nc.dram_tensor("out", [QSH, D], f32, kind="ExternalOutput").ap()

    with TileContext(nc) as tc:
        with (
            tc.tile_pool(name="const", bufs=1) as const_pool,
            tc.tile_pool(name="w", bufs=1) as w_pool,
            tc.tile_pool(name="qnat", bufs=1) as qnat_pool,
            tc.tile_pool(name="qT", bufs=1) as qT_pool,
            tc.tile_pool(name="nat", bufs=8) as nat_pool,
            tc.tile_pool(name="rT", bufs=4) as rT_pool,
            tc.tile_pool(name="proj", bufs=1) as proj_pool,
            tc.tile_pool(name="vp", bufs=1) as vp_pool,
            tc.tile_pool(name="exp", bufs=6) as exp_pool,
            tc.tile_pool(name="fin", bufs=1) as fin_pool,
            tc.tile_pool(name="tpb", bufs=2, space="PSUM") as tpb_psum,
            tc.tile_pool(name="pj", bufs=1, space="PSUM") as pj_psum,
            tc.tile_pool(name="sc", bufs=2, space="PSUM") as sc_psum,
            tc.tile_pool(name="acc", bufs=1, space="PSUM") as acc_psum,
        ):
            identb = const_pool.tile([128, 128], bf16)
            make_identity(nc, identb[:])
            identf = const_pool.tile([128, 128], f32, tag="identf")
            make_identity(nc, identf[:])
            ones_sb = const_pool.tile([128, 1], bf16, tag="ones")
            nc.vector.memset(ones_sb[:], 1.0)

            w_sb = {}
            for name, wd in (("wq", wq_d), ("wk", wk_d), ("wv", wv_d)):
                t = w_pool.tile([128, HC * D], bf16, tag=name, name=name)
                nc.gpsimd.dma_start(
                    out=t[:].rearrange("p (c d) -> p c d", c=HC),
                    in_=wd.rearrange("(c p) d -> p c d", p=128),
                )
                w_sb[name] = t
            bq_sb = const_pool.tile([D, 1], f32, tag="bq")
            bk_sb = const_pool.tile([D, 1], f32, tag="bk")
            bv_sb = const_pool.tile([D, 1], f32, tag="bv")
            nc.sync.dma_start(out=bq_sb[:], in_=bq_d[:])
            nc.sync.dma_start(out=bk_sb[:], in_=bk_d[:])
            nc.sync.dma_start(out=bv_sb[:], in_=bv_d[:])

            def transpose_range(dst, src, jcnt):
                for hp in range(HC // 2):
                    ps = tpb_psum.tile([128, 1024], bf16, tag="tpb", name="psb")
                    for half in range(2):
                        hc = hp * 2 + half
                        for s in range(jcnt):
                            nc.tensor.transpose(
                                ps[:, half * 512 + s * 128 : half * 512 + (s + 1) * 128],
                                src[:, s * H + hc * 128 : s * H + (hc + 1) * 128],
                                identb[:],
                            )
                    nc.vector.tensor_copy(dst[:, hp * 1024 : (hp + 1) * 1024], ps[:])

            # ---- Q ----
            q_nat = qnat_pool.tile([128, 8 * H], bf16)
            for ai in range(2):
                nc.gpsimd.dma_start(
                    out=q_nat[:, ai * 4 * H : (ai + 1) * 4 * H],
                    in_=q_d.rearrange("(p a j) h -> a p (j h)", a=2, j=4)[ai],
                )
            qT = [qT_pool.tile([128, QSH], bf16, tag=f"qT{h}", name=f"qT{h}")
                  for h in range(HC)]
            for hc in range(HC):
                ps = tpb_psum.tile([128, 1024], bf16, tag="tpb", name="psb")
                for qc in range(8):
                    nc.tensor.transpose(
                        ps[:, qc * 128 : (qc + 1) * 128],
                        q_nat[:, qc * H + hc * 128 : qc * H + (hc + 1) * 128],
                        identb[:],
                    )
                nc.vector.tensor_copy(qT[hc][:], ps[:])
            qprojT = proj_pool.tile([D, QSH], bf16, tag="qprojT")
            for qn in range(QSH // 512):
                ps = pj_psum.tile([D, 512], f32, tag="pj", name="pjq")
                for hc in range(HC):
                    nc.tensor.matmul(
                        ps[:],
                        w_sb["wq"][:, hc * D : (hc + 1) * D],
                        qT[hc][:, qn * 512 : (qn + 1) * 512],
                        start=(hc == 0), stop=(hc == HC - 1),
                    )
                nc.scalar.activation(
                    qprojT[:, qn * 512 : (qn + 1) * 512], ps[:],
                    AF.Identity, bias=bq_sb[:], scale=0.125,
                )

            # ---- V ----
            vprojT = proj_pool.tile([D, L], bf16, tag="vprojT")
            vp = vp_pool.tile([128, (L // 128) * 65], bf16, tag="vp")
            for rng in range(NRNG):
                v_nat = nat_pool.tile([128, 4 * H], bf16, tag="nat",
                                      name=f"vnat{rng}")
                nc.gpsimd.dma_start(
                    out=v_nat[:],
                    in_=v_d.rearrange("(r p j) h -> r p (j h)", p=128, j=4)[rng],
                )
                vT = rT_pool.tile([128, HC * 512], bf16, tag="rT",
                                  name=f"vT{rng}")
                transpose_range(vT, v_nat, 4)
                ps = pj_psum.tile([D, 512], f32, tag="pj", name="pjv")
                for hc in range(HC):
                    nc.tensor.matmul(
                        ps[:], w_sb["wv"][:, hc * D : (hc + 1) * D],
                        vT[:, hc * 512 : (hc + 1) * 512],
                        start=(hc == 0), stop=(hc == HC - 1),
                    )
                vs = vprojT[:, rng * 512 : (rng + 1) * 512]
                nc.scalar.activation(vs, ps[:], AF.Identity, bias=bv_sb[:])
                psv = tpb_psum.tile([128, 512], bf16, tag="tpv", name="psv",
                                    bufs=1)
                for s in range(4):
                    nc.tensor.transpose(
                        psv[:, s * 128 : s * 128 + D],
                        vs[:, s * 128 : (s + 1) * 128],
                        identb[0:D, 0:D],
                    )
                for s in range(4):
                    kc = rng * 4 + s
                    nc.vector.tensor_copy(
                        vp[:, kc * 65 : kc * 65 + 64],
                        psv[:, s * 128 : s * 128 + D],
                    )
                    nc.vector.tensor_copy(
                        vp[:, kc * 65 + 64 : kc * 65 + 65], ones_sb[:]
                    )

            # ---- K + scores + attnV ----
            kprojT = proj_pool.tile([D, L], bf16, tag="kprojT")
            outT_ps = acc_psum.tile([65, QSH], f32)
            for rng in range(NRNG):
                k_nat = nat_pool.tile([128, 4 * H], bf16, tag="nat",
                                      name=f"knat{rng}")
                nc.gpsimd.dma_start(
                    out=k_nat[:],
                    in_=k_d.rearrange("(r p j) h -> r p (j h)", p=128, j=4)[rng],
                )
                kT = rT_pool.tile([128, HC * 512], bf16, tag="rT",
                                  name=f"kT{rng}")
                transpose_range(kT, k_nat, 4)
                ps = pj_psum.tile([D, 512], f32, tag="pj", name="pjk")
                for hc in range(HC):
                    nc.tensor.matmul(
                        ps[:], w_sb["wk"][:, hc * D : (hc + 1) * D],
                        kT[:, hc * 512 : (hc + 1) * 512],
                        start=(hc == 0), stop=(hc == HC - 1),
                    )
                kslice = kprojT[:, rng * 512 : (rng + 1) * 512]
                nc.scalar.activation(kslice, ps[:], AF.Identity, bias=bk_sb[:])

                for s in range(4):
                    kc = rng * 4 + s
                    e = exp_pool.tile([128, QSH], bf16, tag="exp")
                    for qn in range(QSH // 512):
                        sc = sc_psum.tile([128, 512], f32, tag="sc")
                        nc.tensor.matmul(
                            sc[:],
                            kprojT[:, kc * 128 : (kc + 1) * 128],
                            qprojT[:, qn * 512 : (qn + 1) * 512],
                            start=True, stop=True,
                        )
                        nc.scalar.activation(
                            e[:, qn * 512 : (qn + 1) * 512], sc[:], AF.Exp
                        )
                    for qn in range(QSH // 512):
                        nc.tensor.matmul(
                            outT_ps[:, qn * 512 : (qn + 1) * 512],
                            vp[:, kc * 65 : (kc + 1) * 65],
                            e[:, qn * 512 : (qn + 1) * 512],
                            start=(kc == 0), stop=(kc == L // 128 - 1),
                            skip_group_check=True,
                        )

            # ---- finalize ----
            outT_sb = fin_pool.tile([65, QSH], f32, tag="outT")
            nc.vector.tensor_copy(outT_sb[:], outT_ps[:])
            out_sb = fin_pool.tile([128, 8 * D], f32, tag="out")
            for qc in range(QSH // 128):
                ps = pj_psum.tile([128, 128], f32, tag="pj", name="pjf")
                nc.tensor.transpose(
                    ps[:, 0:65],
                    outT_sb[:, qc * 128 : (qc + 1) * 128],
                    identf[0:65, 0:65],
                )
                recip = fin_pool.tile([128, 1], f32, tag="recip")
                nc.vector.reciprocal(recip[:], ps[:, 64:65])
                nc.vector.tensor_scalar_mul(
                    out_sb[:, qc * D : (qc + 1) * D], ps[:, 0:D], recip[:]
                )
            nc.sync.dma_start(
                out=out_d.rearrange("(p j) d -> p j d", j=8),
                in_=out_sb[:].rearrange("p (j d) -> p j d", j=8),
            )

    nc.compile()
    return nc


_NC_CACHE = None


def _get_nc():
    global _NC_CACHE
    if _NC_CACHE is None:
        _NC_CACHE = build_bass()
    return _NC_CACHE


def _make_in_maps(inputs):
    query = np.ascontiguousarray(np.asarray(inputs["query"], np.float32))
    key = np.ascontiguousarray(np.asarray(inputs["key"], np.float32))
    value = np.ascontiguousarray(np.asarray(inputs["value"], np.float32))
    wq = np.ascontiguousarray(np.asarray(inputs["Wq"], np.float32))
    wk = np.ascontiguousarray(np.asarray(inputs["Wk"], np.float32))
    wv = np.ascontiguousarray(np.asarray(inputs["Wv"], np.float32))
    bq8 = (np.asarray(inputs["bq"], np.float32) / 8.0).reshape(D, 1)
    bk = np.asarray(inputs["bk"], np.float32).reshape(D, 1).copy()
    bv = np.asarray(inputs["bv"], np.float32).reshape(D, 1).copy()
    in_maps = []
    for c in range(NCORES):
        b, half = divmod(c, 2)
        in_maps.append(
            {
                "q": query[b, half * QSH : (half + 1) * QSH],
                "k": key[b],
                "v": value[b],
                "wq": wq,
                "wk": wk,
                "wv": wv,
                "bq8": bq8,
                "bk": bk,
                "bv": bv,
            }
        )
    return in_maps


def kernel(query, key, value, Wq, bq, Wk, bk, Wv, bv):
    from concourse.bass_utils import run_bass_kernel_spmd

    in_maps = _make_in_maps(
        dict(query=query, key=key, value=value, Wq=Wq, bq=bq, Wk=Wk, bk=bk,
             Wv=Wv, bv=bv)
    )
    nc = _get_nc()
    try:
        res = run_bass_kernel_spmd(nc, in_maps, list(range(NCORES)))
    except Exception:
        res = run_bass_kernel_spmd(nc, in_maps, list(range(NCORES)))
    out = np.empty((N, L, D), np.float32)
    for c in range(NCORES):
        b, half = divmod(c, 2)
        out[b, half * QSH : (half + 1) * QSH] = res.results[c]["out"]
    return out



# revision 5
# speedup vs baseline: 1.6866x; 1.6866x over previous
"""v5: attention head on 8 trn2 NeuronCores, no collectives.

Sharding: core c handles batch b=c//2 and K/V-half j=c%2. Each core computes
unnormalized attention of the batch's FULL query block (2048 rows) against its
1024-row K/V half; the softmax numerator/denominator halves combine linearly on
the host (out = (numA+numB)/(denA+denB)).

Host-side preprocessing (layout/dtype only, no model FLOPs): q/k/v are cast to
bf16 and pre-transposed to the [H, seq] layout the PE contraction needs, so the
device does zero PE transposes of inputs and reads half the HBM bytes. The 1/8
score scale is folded into Wq/bq on the host.

Device per core: 8.5MB HWDGE loads; projections (PE, W stationary);
scores [k,q] orientation; exp on ACT (PSUM->SBUF bf16); attnV with
V-natural stationary carrying a ones column (row 64 = softmax denominator);
unnormalized [65, 2048] f32 result DMA'd out.
"""

import sys

if "/opt/trn_rl_repo" not in sys.path:
    sys.path.insert(0, "/opt/trn_rl_repo")

import numpy as np
import ml_dtypes

N, L, H, D = 4, 2048, 1024, 64
NCORES = 8
HC = H // 128  # 8 h-chunks
KH = L // 2  # 1024 rows of K/V per core
KC = KH // 128  # 8 k-chunks per core


def build_bass():
    import concourse.mybir as mybir
    from concourse import bacc
    from concourse.masks import make_identity
    from concourse.tile import TileContext

    f32 = mybir.dt.float32
    bf16 = mybir.dt.bfloat16
    AF = mybir.ActivationFunctionType

    nc = bacc.Bacc("TRN2", target_bir_lowering=False, debug=False)
    qt_d = nc.dram_tensor("qt", [128, 16 * H], bf16, kind="ExternalInput").ap()
    kt_d = nc.dram_tensor("kt", [128, 8 * H], bf16, kind="ExternalInput").ap()
    vt_d = nc.dram_tensor("vt", [128, 8 * H], bf16, kind="ExternalInput").ap()
    w_d = nc.dram_tensor("w", [128, 3 * HC * D], bf16, kind="ExternalInput").ap()
    b_d = nc.dram_tensor("b", [D, 3], f32, kind="ExternalInput").ap()
    out_d = nc.dram_tensor("out", [D + 1, L], f32, kind="ExternalOutput").ap()

    with TileContext(nc) as tc:
        with (
            tc.tile_pool(name="io", bufs=1) as io_pool,
            tc.tile_pool(name="proj", bufs=1) as proj_pool,
            tc.tile_pool(name="e", bufs=4) as e_pool,
            tc.tile_pool(name="ps", bufs=2, space="PSUM") as ps_pool,
            tc.tile_pool(name="acc", bufs=1, space="PSUM") as acc_pool,
        ):
            w_sb = io_pool.tile([128, 3 * HC * D], bf16, tag="w")
            b_sb = io_pool.tile([D, 3], f32, tag="b")
            kt_sb = io_pool.tile([128, 8 * H], bf16, tag="kt")
            vt_sb = io_pool.tile([128, 8 * H], bf16, tag="vt")
            qt_sb = io_pool.tile([128, 16 * H], bf16, tag="qt")
            nc.sync.dma_start(out=w_sb[:], in_=w_d[:])
            nc.sync.dma_start(out=b_sb[:], in_=b_d[:])
            nc.sync.dma_start(out=kt_sb[:], in_=kt_d[:])
            nc.sync.dma_start(out=vt_sb[:], in_=vt_d[:])
            for qnp in range(2):
                nc.sync.dma_start(
                    out=qt_sb[:, qnp * 8 * H : (qnp + 1) * 8 * H],
                    in_=qt_d[:, qnp * 8 * H : (qnp + 1) * 8 * H],
                )

            identf = io_pool.tile([64, 64], f32, tag="identf")
            make_identity(nc, identf[:])

            kprojT = proj_pool.tile([D, KH], bf16, tag="kprojT")
            qprojT = proj_pool.tile([D, L], bf16, tag="qprojT")
            vp = proj_pool.tile([128, KC * (D + 1)], bf16, tag="vp")
            outT_sb = proj_pool.tile([D + 1, L], f32, tag="outT")

            # ---- K projection: kprojT[d, s] = sum_h Wk[h,d] * kT[h, s] ----
            psk = ps_pool.tile([128, 1024], f32, tag="ps", name="psk")
            for hc in range(HC):
                wslice = w_sb[:, (HC + hc) * D : (HC + hc + 1) * D]
                for sn in range(2):
                    nc.tensor.matmul(
                        psk[0:D, sn * 512 : (sn + 1) * 512],
                        wslice,
                        kt_sb[:, hc * KH + sn * 512 : hc * KH + (sn + 1) * 512],
                        start=(hc == 0), stop=(hc == HC - 1),
                    )
            nc.vector.tensor_scalar_add(kprojT[:], psk[0:D, :], b_sb[:, 1:2])

            # ---- V projection + re-transpose to vp [k-part, d] + ones col ----
            psv = ps_pool.tile([128, 1024], f32, tag="ps", name="psv")
            for hc in range(HC):
                wslice = w_sb[:, (2 * HC + hc) * D : (2 * HC + hc + 1) * D]
                for sn in range(2):
                    nc.tensor.matmul(
                        psv[0:D, sn * 512 : (sn + 1) * 512],
                        wslice,
                        vt_sb[:, hc * KH + sn * 512 : hc * KH + (sn + 1) * 512],
                        start=(hc == 0), stop=(hc == HC - 1),
                    )
            vprojT = proj_pool.tile([D, KH], f32, tag="vprojT")
            nc.vector.tensor_scalar_add(vprojT[:], psv[0:D, :], b_sb[:, 2:3])
            pst = ps_pool.tile([128, 1024], f32, tag="ps", name="pst")
            for s in range(KC):
                nc.tensor.transpose(
                    pst[:, s * 128 : s * 128 + D],
                    vprojT[:, s * 128 : (s + 1) * 128],
                    identf[:],
                )
            for s in range(KC):
                nc.vector.tensor_copy(
                    vp[:, s * (D + 1) : s * (D + 1) + D],
                    pst[:, s * 128 : s * 128 + D],
                )
            nc.vector.memset(vp[:, D :: D + 1], 1.0)

            # ---- Q projection (scale folded into Wq/bq on host) ----
            for qnp in range(2):
                psq = ps_pool.tile([128, 1024], f32, tag="ps", name=f"psq{qnp}")
                for hc in range(HC):
                    wslice = w_sb[:, hc * D : (hc + 1) * D]
                    for qi in range(2):
                        qn = qnp * 2 + qi
                        nc.tensor.matmul(
                            psq[0:D, qi * 512 : (qi + 1) * 512],
                            wslice,
                            qt_sb[:, qn * HC * 512 + hc * 512 : qn * HC * 512 + (hc + 1) * 512],
                            start=(hc == 0), stop=(hc == HC - 1),
                        )
                nc.vector.tensor_scalar_add(
                    qprojT[:, qnp * 1024 : (qnp + 1) * 1024], psq[0:D, :], b_sb[:, 0:1]
                )

            # ---- scores -> exp -> attnV (accumulate with ones column) ----
            acc = acc_pool.tile([D + 1, L], f32, tag="acc")
            for qnp in range(2):
                for kc in range(KC):
                    sct = ps_pool.tile([128, 1024], f32, tag="ps", name=f"sc{qnp}_{kc}")
                    for qi in range(2):
                        qn = qnp * 2 + qi
                        nc.tensor.matmul(
                            sct[:, qi * 512 : (qi + 1) * 512],
                            kprojT[:, kc * 128 : (kc + 1) * 128],
                            qprojT[:, qn * 512 : (qn + 1) * 512],
                            start=True, stop=True,
                        )
                    et = e_pool.tile([128, 1024], bf16, tag="e")
                    nc.scalar.activation(et[:], sct[:], AF.Exp)
                    for qi in range(2):
                        qn = qnp * 2 + qi
                        nc.tensor.matmul(
                            acc[:, qn * 512 : (qn + 1) * 512],
                            vp[:, kc * (D + 1) : (kc + 1) * (D + 1)],
                            et[:, qi * 512 : (qi + 1) * 512],
                            start=(kc == 0), stop=(kc == KC - 1),
                            skip_group_check=True,
                        )

            # ---- epilogue: unnormalized [65, 2048] out ----
            nc.vector.tensor_copy(outT_sb[:], acc[:])
            nc.sync.dma_start(out=out_d[:], in_=outT_sb[:])

    nc.compile()
    return nc


_NC_CACHE = None


def _get_nc():
    global _NC_CACHE
    if _NC_CACHE is None:
        _NC_CACHE = build_bass()
    return _NC_CACHE


def _make_in_maps(inputs):
    bf16 = ml_dtypes.bfloat16
    q = np.asarray(inputs["query"], np.float32)
    k = np.asarray(inputs["key"], np.float32)
    v = np.asarray(inputs["value"], np.float32)
    Wq = np.asarray(inputs["Wq"], np.float32) * 0.125
    bq = np.asarray(inputs["bq"], np.float32) * 0.125
    Wk = np.asarray(inputs["Wk"], np.float32)
    bk = np.asarray(inputs["bk"], np.float32)
    Wv = np.asarray(inputs["Wv"], np.float32)
    bv = np.asarray(inputs["bv"], np.float32)

    def packw(W):  # [1024, 64] -> [128, 8*64], hc-major per partition
        return W.reshape(HC, 128, D).transpose(1, 0, 2).reshape(128, HC * D)

    wcat = np.concatenate([packw(Wq), packw(Wk), packw(Wv)], axis=1).astype(bf16)
    bcat = np.stack([bq, bk, bv], axis=1).astype(np.float32)

    def tr(x):  # [S, 1024] -> [128, 8*S]: [p, hc*S + s] = x[s, hc*128+p]
        S = x.shape[0]
        return np.ascontiguousarray(
            x.reshape(S, HC, 128).transpose(2, 1, 0)
        ).reshape(128, HC * S).astype(bf16)

    in_maps = []
    for c in range(NCORES):
        b, j = divmod(c, 2)
        qb = q[b]  # [2048, 1024]
        # [p, qn*4096 + hc*512 + s] = qb[qn*512+s, hc*128+p]
        qT = np.ascontiguousarray(
            qb.reshape(4, 512, HC, 128).transpose(3, 0, 2, 1)
        ).reshape(128, 16 * H).astype(bf16)
        kT = tr(k[b, j * KH : (j + 1) * KH])
        vT = tr(v[b, j * KH : (j + 1) * KH])
        in_maps.append({"qt": qT, "kt": kT, "vt": vT, "w": wcat, "b": bcat})
    return in_maps


def kernel(query, key, value, Wq, bq, Wk, bk, Wv, bv):
    from concourse.bass_utils import run_bass_kernel_spmd

    in_maps = _make_in_maps(
        dict(query=query, key=key, value=value, Wq=Wq, bq=bq, Wk=Wk, bk=bk,
             Wv=Wv, bv=bv)
    )
    nc = _get_nc()
    try:
        res = run_bass_kernel_spmd(nc, in_maps, list(range(NCORES)))
    except Exception:
        res = run_bass_kernel_spmd(nc, in_maps, list(range(NCORES)))
    out = np.empty((N, L, D), np.float32)
    for b in range(N):
        o0 = np.asarray(res.results[2 * b]["out"], np.float32)
        o1 = np.asarray(res.results[2 * b + 1]["out"], np.float32)
        num = o0[0:D] + o1[0:D]  # [64, 2048]
        den = o0[D] + o1[D]  # [2048]
        out[b] = (num / den).T
    return out
